# revision 7
# baseline (speedup 1.0000x reference)
"""RPE multi-head attention kernel for Trainium2, 8 NeuronCores.

Problem: B=2, S=2048, H=8, DEPTH=64, D_MODEL=512.
  Q_c = emb_x @ Wq + bq ; Q_r = pe @ Wq + bq ; K_c = emb_x @ Wk + bk
  K_r = pe @ Wk + bk ; V = emb_x @ Wv + bv
  logits = (Q_c K_c^T + Q_r K_c^T + Q_c K_r^T) / 8
  attn = softmax(logits) ; z = attn @ V ; out = concat_heads(z) @ Wo + bo
Outputs: (out [2,2048,512], attn [2,8,2048,2048]) both f32.

Sharding: 16 (batch, head) pairs over 8 cores -> core c handles batch c//4,
heads {2*(c%4), 2*(c%4)+1}. Projection weights host-sliced per head pair.
The 3-term logits fold into one K=128 contraction via stacked operands:
QQ = [Q_c^T; Q_r^T], KK = [(K_c+K_r)^T; K_c^T] (both [128, S]):
  natural  S[q,k]   : lhsT = QQ[:, qtile], rhs = KK   (for attn output)
  transposed S^T[k,q]: lhsT = KK[:, ktile], rhs = QQ  (for attn @ V)
Matmuls run in f32r (TF32-like, full PE rate) except attn@V (bf16 — only
feeds `out`, whose tolerance is looser than the 128MiB attn output).
f32r cannot target PSUM partition-offset destinations, so stacked upper
halves are assembled via SBUF->SBUF DMA (the only partition-crossing path).
Softmax denominators come free from ACT accum_out on the natural exp pass.
Host sums the 4 per-core partial outputs per batch and adds bo.
"""
import sys

if "/opt/trn_rl_repo" not in sys.path:
    sys.path.insert(0, "/opt/trn_rl_repo")

import numpy as np

import concourse.bacc as bacc
import concourse.mybir as mybir
import concourse.tile as tile
from concourse import bass_utils
from concourse.masks import make_identity

F32 = mybir.dt.float32
F32R = mybir.dt.float32r
BF16 = mybir.dt.bfloat16
EXP = mybir.ActivationFunctionType.Exp

S = 2048
D = 512
QT = S // 128   # 16 q/k tiles of 128
NC = S // 512   # 4 moving chunks of 512
DC = D // 128   # 4 d_model chunks

_CACHED_NC = None


def _build():
    nc = bacc.Bacc("TRN2", target_bir_lowering=False, debug=False,
                   enable_asserts=False, num_devices=8)

    x_d = nc.dram_tensor("x", [S, D], F32, kind="ExternalInput").ap()
    pe_d = nc.dram_tensor("pe", [S, D], F32, kind="ExternalInput").ap()
    wq_d = nc.dram_tensor("wq", [D, 128], F32, kind="ExternalInput").ap()
    wk_d = nc.dram_tensor("wk", [D, 128], F32, kind="ExternalInput").ap()
    wv_d = nc.dram_tensor("wv", [D, 128], F32, kind="ExternalInput").ap()
    wo_d = nc.dram_tensor("wo", [128, D], F32, kind="ExternalInput").ap()
    bias_d = nc.dram_tensor("bias", [128, 7], F32, kind="ExternalInput").ap()
    attn_d = nc.dram_tensor("attn_sh", [2, S, S], F32, kind="ExternalOutput").ap()
    outp_d = nc.dram_tensor("outp", [S, D], F32, kind="ExternalOutput").ap()

    with tile.TileContext(nc) as tc:
        with tc.tile_pool(name="const", bufs=1) as cp, \
             tc.tile_pool(name="main", bufs=1) as mp:
            ident = cp.tile([128, 128], F32, tag="ident")
            make_identity(nc, ident[:])
            bias_sb = cp.tile([128, 7], F32, tag="bias")
            nc.sync.dma_start(out=bias_sb[:], in_=bias_d)
            # weights, DMA-cast to f32r; layout [128(p of d-chunk), c, j]
            wq_sb = cp.tile([128, D], F32R, tag="wq")
            wk_sb = cp.tile([128, D], F32R, tag="wk")
            wv_sb = cp.tile([128, D], F32R, tag="wv")
            for t, d in ((wq_sb, wq_d), (wk_sb, wk_d), (wv_sb, wv_d)):
                nc.gpsimd.dma_start(
                    out=t[:].rearrange("p (c j) -> p c j", j=128),
                    in_=d.rearrange("(c p) j -> p c j", p=128))
            wo0_sb = cp.tile([64, D], F32R, tag="wo0")
            wo1_sb = cp.tile([64, D], F32R, tag="wo1")
            nc.gpsimd.dma_start(out=wo0_sb[:], in_=wo_d[0:64, :])
            nc.gpsimd.dma_start(out=wo1_sb[:], in_=wo_d[64:128, :])

            # Persistent mid-size tensors
            QQ = [mp.tile([128, S], F32R, tag=f"qq{h}", name=f"qq{h}")
                  for h in range(2)]
            KK = [mp.tile([128, S], F32R, tag=f"kk{h}", name=f"kk{h}")
                  for h in range(2)]
            V4 = [mp.tile([128, 512], BF16, tag=f"v4{g}", name=f"v4{g}")
                  for g in range(4)]
            zz0_sb = mp.tile([64, S], F32R, tag="zz0")
            zz1_sb = mp.tile([64, S], F32R, tag="zz1")
            Z_sb = [mp.tile([128, QT], F32, tag=f"z{h}", name=f"zs{h}")
                    for h in range(2)]
            R_sb = [mp.tile([128, QT], F32, tag=f"r{h}", name=f"rs{h}")
                    for h in range(2)]

            # ---------------- Preamble: x^T, pe^T ----------------
            with tc.tile_pool(name="xt", bufs=1) as xtp:
                xT = [xtp.tile([128, S], F32R, tag=f"xt{c}", name=f"xt{c}")
                      for c in range(DC)]
                peT = [xtp.tile([128, S], F32R, tag=f"pet{c}", name=f"pet{c}")
                       for c in range(DC)]

                with tc.tile_pool(name="nat", bufs=6) as natp, \
                     tc.tile_pool(name="trp", bufs=2, space="PSUM") as trp:
                    for dst, src in ((xT, x_d), (peT, pe_d)):
                        for g in range(4):          # groups of 4 q-tiles
                            ptr = [trp.tile([128, 512], F32, tag=f"tr{c}",
                                            name=f"tr{c}") for c in range(DC)]
                            for i in range(4):      # q-tile within group
                                qt = g * 4 + i
                                nat = natp.tile([128, D], F32, tag="nat")
                                nc.sync.dma_start(
                                    out=nat[:], in_=src[qt * 128:(qt + 1) * 128, :])
                                for c in range(DC):
                                    nc.tensor.transpose(
                                        ptr[c][:, i * 128:(i + 1) * 128],
                                        nat[:, c * 128:(c + 1) * 128], ident[:])
                            for c in range(DC):
                                nc.vector.tensor_copy(
                                    dst[c][:, g * 512:(g + 1) * 512], ptr[c][:])

                # ---------------- Projections ----------------
                # One [64, S] base-0 psum tile per stacked half; lower halves
                # DVE-copy straight in, upper halves stage + SBUF->SBUF DMA.
                def proj64(w_sb, h, rhs_list, accumulate, dst_ap, bias_col,
                           stage_pool):
                    ps = stage_pool.tile([64, S], F32, tag="pj", name="pj")
                    groups = rhs_list if accumulate else [rhs_list]
                    for gi, rhs_src in enumerate(groups):
                        for c in range(DC):
                            for n in range(NC):
                                nc.tensor.matmul(
                                    ps[:, n * 512:(n + 1) * 512],
                                    lhsT=w_sb[:, c * 128 + h * 64:
                                              c * 128 + h * 64 + 64],
                                    rhs=rhs_src[c][:, n * 512:(n + 1) * 512],
                                    start=(gi == 0 and c == 0),
                                    stop=(gi == len(groups) - 1 and c == DC - 1))
                    nc.vector.tensor_scalar_add(
                        dst_ap, ps[:], bias_sb[0:64, bias_col:bias_col + 1])

                with tc.tile_pool(name="pjp", bufs=2, space="PSUM") as pjp:
                    stg = mp.tile([64, S], F32R, tag="stg")
                    for h in range(2):
                        # Q_c -> QQ[h][0:64]
                        proj64(wq_sb, h, xT, False, QQ[h][0:64, :], h, pjp)
                        # Q_r -> stage -> QQ[h][64:128]
                        proj64(wq_sb, h, peT, False, stg[:], h, pjp)
                        nc.sync.dma_start(out=QQ[h][64:128, :], in_=stg[:])
                        # K_sum -> KK[h][0:64]
                        proj64(wk_sb, h, [xT, peT], True, KK[h][0:64, :],
                               2 + 2 * h, pjp)
                        # K_c -> stage -> KK[h][64:128]
                        proj64(wk_sb, h, xT, False, stg[:], 3 + 2 * h, pjp)
                        nc.sync.dma_start(out=KK[h][64:128, :], in_=stg[:])

                    # V^T [128(d both heads), S]
                    psv = pjp.tile([128, S], F32, tag="pj")
                    for c in range(DC):
                        for n in range(NC):
                            nc.tensor.matmul(
                                psv[:, n * 512:(n + 1) * 512],
                                lhsT=wv_sb[:, c * 128:(c + 1) * 128],
                                rhs=xT[c][:, n * 512:(n + 1) * 512],
                                start=(c == 0), stop=(c == DC - 1))
                    vt_sb = mp.tile([128, S], F32, tag="vt")
                    nc.vector.tensor_scalar_add(
                        vt_sb[:], psv[:], bias_sb[:, 6:7])

                # V natural (bf16): transpose V^T k-tiles
                with tc.tile_pool(name="vtr", bufs=2, space="PSUM") as vtrp:
                    for g in range(4):
                        pv = vtrp.tile([128, 512], F32, tag="vtr")
                        for i in range(4):
                            kt = g * 4 + i
                            nc.tensor.transpose(
                                pv[:, i * 128:(i + 1) * 128],
                                vt_sb[:, kt * 128:(kt + 1) * 128], ident[:])
                        nc.vector.tensor_copy(V4[g][:], pv[:])

            # ---- Interleaved softmax: A (natural S -> attn) + B (S^T -> z) --
            # Half-tiles [128,1024] ping-pong across two single-buffered PSUM
            # pools + the zz accumulator: 2+2+4 = 8 banks exactly. PE stays
            # dense (HAM warm); ACT paces the loop.
            with tc.tile_pool(name="psA", bufs=1, space="PSUM") as psA, \
                 tc.tile_pool(name="psB", bufs=1, space="PSUM") as psB, \
                 tc.tile_pool(name="zzp", bufs=1, space="PSUM") as zzp, \
                 tc.tile_pool(name="attn", bufs=6) as ap, \
                 tc.tile_pool(name="et", bufs=2) as etp:
                zz = zzp.tile([128, S], F32, tag="zz")
                Z2 = [mp.tile([128, 2 * QT], F32, tag=f"z2{h}", name=f"z2{h}")
                      for h in range(2)]
                for h in range(2):
                    for qt in range(QT):
                        kt = qt
                        at = ap.tile([128, S], F32, tag="at")
                        et = etp.tile([128, S], BF16, tag="et")
                        for half in range(2):
                            lo = half * 1024
                            # A: S[qtile, khalf] -> exp -> attn tile half
                            psa = psA.tile([128, 1024], F32, tag="sA")
                            for j in range(2):
                                n = 2 * half + j
                                nc.tensor.matmul(
                                    psa[:, j * 512:(j + 1) * 512],
                                    lhsT=QQ[h][:, qt * 128:(qt + 1) * 128],
                                    rhs=KK[h][:, n * 512:(n + 1) * 512],
                                    start=True, stop=True)
                            nc.scalar.activation(
                                at[:, lo:lo + 1024], psa[:], EXP, scale=0.125,
                                accum_out=Z2[h][:, 2 * qt + half:2 * qt + half + 1])
                            # B: S^T[ktile, qhalf] -> exp -> E^T half -> AV
                            psb = psB.tile([128, 1024], F32, tag="sB")
                            for j in range(2):
                                n = 2 * half + j
                                nc.tensor.matmul(
                                    psb[:, j * 512:(j + 1) * 512],
                                    lhsT=KK[h][:, kt * 128:(kt + 1) * 128],
                                    rhs=QQ[h][:, n * 512:(n + 1) * 512],
                                    start=True, stop=True)
                            nc.scalar.activation(
                                et[:, lo:lo + 1024], psb[:], EXP, scale=0.125)
                            for j in range(2):
                                n = 2 * half + j
                                nc.tensor.matmul(
                                    zz[h * 64:(h + 1) * 64, n * 512:(n + 1) * 512],
                                    lhsT=V4[kt // 4][:, (kt % 4) * 128 + h * 64:
                                                     (kt % 4) * 128 + h * 64 + 64],
                                    rhs=et[:, n * 512:(n + 1) * 512],
                                    start=(kt == 0), stop=(kt == QT - 1))
                        # combine half-sums, normalize, store attn tile
                        nc.vector.tensor_add(
                            Z_sb[h][:, qt:qt + 1],
                            Z2[h][:, 2 * qt:2 * qt + 1],
                            Z2[h][:, 2 * qt + 1:2 * qt + 2])
                        nc.vector.reciprocal(
                            R_sb[h][:, qt:qt + 1], Z_sb[h][:, qt:qt + 1])
                        nc.vector.tensor_scalar_mul(
                            at[:], at[:], R_sb[h][:, qt:qt + 1])
                        nc.sync.dma_start(
                            out=attn_d[h, qt * 128:(qt + 1) * 128, :], in_=at[:])
                zstg = mp.tile([128, S], F32R, tag="zstg")
                nc.vector.tensor_copy(zz0_sb[:], zz[0:64, :])
                nc.vector.tensor_copy(zstg[64:128, :], zz[64:128, :])
                nc.sync.dma_start(out=zz1_sb[:], in_=zstg[64:128, :])

            # ---------------- Out projection (all base-0, f32r) -----------
            with tc.tile_pool(name="psO", bufs=2, space="PSUM") as psO, \
                 tc.tile_pool(name="ot", bufs=3) as otp:
                for qt in range(QT):
                    po = [psO.tile([128, D], F32, tag=f"po{h}", name=f"po{h}")
                          for h in range(2)]
                    for h, (zsb, wsb) in enumerate(
                            ((zz0_sb, wo0_sb), (zz1_sb, wo1_sb))):
                        nc.tensor.matmul(
                            po[h][:],
                            lhsT=zsb[:, qt * 128:(qt + 1) * 128],
                            rhs=wsb[:], start=True, stop=True)
                    t0 = otp.tile([128, D], F32, tag="t0")
                    t1 = otp.tile([128, D], F32, tag="t1")
                    nc.vector.tensor_scalar_mul(t0[:], po[0][:],
                                                R_sb[0][:, qt:qt + 1])
                    nc.vector.tensor_scalar_mul(t1[:], po[1][:],
                                                R_sb[1][:, qt:qt + 1])
                    ot = otp.tile([128, D], F32, tag="ot")
                    nc.vector.tensor_add(ot[:], t0[:], t1[:])
                    nc.sync.dma_start(
                        out=outp_d[qt * 128:(qt + 1) * 128, :], in_=ot[:])

    nc.compile()
    return nc


def _get_nc():
    global _CACHED_NC
    if _CACHED_NC is None:
        _CACHED_NC = _build()
    return _CACHED_NC


def _in_maps(emb_x, pe_rel_t, Wq, bq, Wk, bk, Wv, bv, Wo, bo):
    maps = []
    for c in range(8):
        b = c // 4
        h0 = 2 * (c % 4)
        sl = slice(64 * h0, 64 * (h0 + 2))
        bq2, bk2, bv2 = bq[sl], bk[sl], bv[sl]
        bias = np.zeros((128, 7), np.float32)
        bias[0:64, 0] = bq2[0:64]          # Q_c/Q_r bias, head 0 of pair
        bias[0:64, 1] = bq2[64:128]        # head 1 of pair
        bias[0:64, 2] = 2.0 * bk2[0:64]    # K_sum bias h0
        bias[0:64, 3] = bk2[0:64]          # K_c bias h0
        bias[0:64, 4] = 2.0 * bk2[64:128]  # K_sum bias h1
        bias[0:64, 5] = bk2[64:128]        # K_c bias h1
        bias[:, 6] = bv2                   # V bias (both heads)
        maps.append({
            "x": np.ascontiguousarray(emb_x[b]),
            "pe": np.ascontiguousarray(pe_rel_t[b]),
            "wq": np.ascontiguousarray(Wq[:, sl]),
            "wk": np.ascontiguousarray(Wk[:, sl]),
            "wv": np.ascontiguousarray(Wv[:, sl]),
            "wo": np.ascontiguousarray(Wo[sl, :]),
            "bias": bias,
        })
    return maps


def _run(in_maps, **kw):
    nc = _get_nc()
    return bass_utils.run_bass_kernel_spmd(nc, in_maps, core_ids=list(range(8)), **kw)


def _assemble(results, bo):
    attn = np.empty((2, 8, S, S), np.float32)
    out = np.empty((2, S, D), np.float32)
    for b in range(2):
        acc = None
        for p in range(4):
            r = results[4 * b + p]
            attn[b, 2 * p:2 * p + 2] = r["attn_sh"]
            acc = r["outp"] if acc is None else acc + r["outp"]
        out[b] = acc + bo[None, :]
    return out, attn


def kernel(emb_x, pe_rel_t, Wq, bq, Wk, bk, Wv, bv, Wo, bo):
    args = [np.asarray(a, np.float32) for a in
            (emb_x, pe_rel_t, Wq, bq, Wk, bk, Wv, bv, Wo, bo)]
    res = _run(_in_maps(*args))
    return _assemble(res.results, args[9])


# revision 8
# speedup vs baseline: 1.0839x; 1.0839x over previous
"""RPE multi-head attention kernel for Trainium2, 8 NeuronCores.

Problem: B=2, S=2048, H=8, DEPTH=64, D_MODEL=512.
  Q_c = emb_x @ Wq + bq ; Q_r = pe @ Wq + bq ; K_c = emb_x @ Wk + bk
  K_r = pe @ Wk + bk ; V = emb_x @ Wv + bv
  logits = (Q_c K_c^T + Q_r K_c^T + Q_c K_r^T) / 8
  attn = softmax(logits) ; z = attn @ V ; out = concat_heads(z) @ Wo + bo
Outputs: (out [2,2048,512], attn [2,8,2048,2048]) both f32.

Sharding: 16 (batch, head) pairs over 8 cores -> core c handles batch c//4,
heads {2*(c%4), 2*(c%4)+1}. Projection weights host-sliced per head pair.
The 3-term logits fold into one K=128 contraction via stacked operands:
QQ = [Q_c^T; Q_r^T], KK = [(K_c+K_r)^T; K_c^T] (both [128, S]):
  natural  S[q,k]   : lhsT = QQ[:, qtile], rhs = KK   (for attn output)
  transposed S^T[k,q]: lhsT = KK[:, ktile], rhs = QQ  (for attn @ V)
Matmuls run in f32r (TF32-like, full PE rate) except attn@V (bf16 — only
feeds `out`, whose tolerance is looser than the 128MiB attn output).
f32r cannot target PSUM partition-offset destinations, so stacked upper
halves are assembled via SBUF->SBUF DMA (the only partition-crossing path).
Softmax denominators come free from ACT accum_out on the natural exp pass.
Host sums the 4 per-core partial outputs per batch and adds bo.
"""
import sys

if "/opt/trn_rl_repo" not in sys.path:
    sys.path.insert(0, "/opt/trn_rl_repo")

import numpy as np

import concourse.bacc as bacc
import concourse.mybir as mybir
import concourse.tile as tile
from concourse import bass_utils
from concourse.masks import make_identity

F32 = mybir.dt.float32
F32R = mybir.dt.float32r
BF16 = mybir.dt.bfloat16
EXP = mybir.ActivationFunctionType.Exp

S = 2048
D = 512
QT = S // 128   # 16 q/k tiles of 128
NC = S // 512   # 4 moving chunks of 512
DC = D // 128   # 4 d_model chunks

_CACHED_NC = None


def _build():
    nc = bacc.Bacc("TRN2", target_bir_lowering=False, debug=False,
                   enable_asserts=False, num_devices=8)

    x_d = nc.dram_tensor("x", [D, S], F32, kind="ExternalInput").ap()
    pe_d = nc.dram_tensor("pe", [D, S], F32, kind="ExternalInput").ap()
    wq_d = nc.dram_tensor("wq", [D, 128], F32, kind="ExternalInput").ap()
    wk_d = nc.dram_tensor("wk", [D, 128], F32, kind="ExternalInput").ap()
    wv_d = nc.dram_tensor("wv", [D, 128], F32, kind="ExternalInput").ap()
    wo_d = nc.dram_tensor("wo", [128, D], F32, kind="ExternalInput").ap()
    bias_d = nc.dram_tensor("bias", [128, 7], F32, kind="ExternalInput").ap()
    attn_d = nc.dram_tensor("attn_sh", [2, S, S], F32, kind="ExternalOutput").ap()
    outp0_d = nc.dram_tensor("outp0", [S, D], F32, kind="ExternalOutput").ap()
    outp1_d = nc.dram_tensor("outp1", [S, D], F32, kind="ExternalOutput").ap()
    zout_d = nc.dram_tensor("zout", [128, 2 * QT], F32, kind="ExternalOutput").ap()

    with tile.TileContext(nc) as tc:
        with tc.tile_pool(name="const", bufs=1) as cp, \
             tc.tile_pool(name="main", bufs=1) as mp:
            ident = cp.tile([128, 128], F32, tag="ident")
            make_identity(nc, ident[:])
            bias_sb = cp.tile([128, 7], F32, tag="bias")
            nc.sync.dma_start(out=bias_sb[:], in_=bias_d)
            # weights, DMA-cast to f32r; layout [128(p of d-chunk), c, j]
            wq_sb = cp.tile([128, D], F32R, tag="wq")
            wk_sb = cp.tile([128, D], F32R, tag="wk")
            wv_sb = cp.tile([128, D], F32R, tag="wv")
            for t, d in ((wq_sb, wq_d), (wk_sb, wk_d), (wv_sb, wv_d)):
                nc.gpsimd.dma_start(
                    out=t[:].rearrange("p (c j) -> p c j", j=128),
                    in_=d.rearrange("(c p) j -> p c j", p=128))
            wo0_sb = cp.tile([64, D], F32R, tag="wo0")
            wo1_sb = cp.tile([64, D], F32R, tag="wo1")
            nc.gpsimd.dma_start(out=wo0_sb[:], in_=wo_d[0:64, :])
            nc.gpsimd.dma_start(out=wo1_sb[:], in_=wo_d[64:128, :])

            # Persistent mid-size tensors
            QQ = [mp.tile([128, S], F32R, tag=f"qq{h}", name=f"qq{h}")
                  for h in range(2)]
            KK = [mp.tile([128, S], F32R, tag=f"kk{h}", name=f"kk{h}")
                  for h in range(2)]
            V4 = [mp.tile([128, 512], BF16, tag=f"v4{g}", name=f"v4{g}")
                  for g in range(4)]
            zz0_sb = mp.tile([64, S], F32R, tag="zz0")
            zz1_sb = mp.tile([64, S], F32R, tag="zz1")
            Z_sb = [mp.tile([128, QT], F32, tag=f"z{h}", name=f"zs{h}")
                    for h in range(2)]
            R_sb = [mp.tile([128, QT], F32, tag=f"r{h}", name=f"rs{h}")
                    for h in range(2)]

            # ------- Preamble: x^T, pe^T arrive pre-transposed from host ----
            with tc.tile_pool(name="xt", bufs=1) as xtp:
                xT = [xtp.tile([128, S], F32R, tag=f"xt{c}", name=f"xt{c}")
                      for c in range(DC)]
                peT = [xtp.tile([128, S], F32R, tag=f"pet{c}", name=f"pet{c}")
                       for c in range(DC)]
                for c in range(DC):
                    nc.gpsimd.dma_start(
                        out=xT[c][:], in_=x_d[c * 128:(c + 1) * 128, :])
                    nc.gpsimd.dma_start(
                        out=peT[c][:], in_=pe_d[c * 128:(c + 1) * 128, :])

                # ---------------- Projections ----------------
                # One [64, S] base-0 psum tile per stacked half; lower halves
                # DVE-copy straight in, upper halves stage + SBUF->SBUF DMA.
                def proj64(w_sb, h, rhs_list, accumulate, dst_ap, bias_col,
                           stage_pool):
                    ps = stage_pool.tile([64, S], F32, tag="pj", name="pj")
                    groups = rhs_list if accumulate else [rhs_list]
                    for gi, rhs_src in enumerate(groups):
                        for c in range(DC):
                            for n in range(NC):
                                nc.tensor.matmul(
                                    ps[:, n * 512:(n + 1) * 512],
                                    lhsT=w_sb[:, c * 128 + h * 64:
                                              c * 128 + h * 64 + 64],
                                    rhs=rhs_src[c][:, n * 512:(n + 1) * 512],
                                    start=(gi == 0 and c == 0),
                                    stop=(gi == len(groups) - 1 and c == DC - 1))
                    nc.vector.tensor_scalar_add(
                        dst_ap, ps[:], bias_sb[0:64, bias_col:bias_col + 1])

                with tc.tile_pool(name="pjp", bufs=2, space="PSUM") as pjp:
                    stg = mp.tile([64, S], F32R, tag="stg")
                    for h in range(2):
                        # Q_c -> QQ[h][0:64]
                        proj64(wq_sb, h, xT, False, QQ[h][0:64, :], h, pjp)
                        # Q_r -> stage -> QQ[h][64:128]
                        proj64(wq_sb, h, peT, False, stg[:], h, pjp)
                        nc.sync.dma_start(out=QQ[h][64:128, :], in_=stg[:])
                        # K_sum -> KK[h][0:64]
                        proj64(wk_sb, h, [xT, peT], True, KK[h][0:64, :],
                               2 + 2 * h, pjp)
                        # K_c -> stage -> KK[h][64:128]
                        proj64(wk_sb, h, xT, False, stg[:], 3 + 2 * h, pjp)
                        nc.sync.dma_start(out=KK[h][64:128, :], in_=stg[:])

                    # V^T [128(d both heads), S]
                    psv = pjp.tile([128, S], F32, tag="pj")
                    for c in range(DC):
                        for n in range(NC):
                            nc.tensor.matmul(
                                psv[:, n * 512:(n + 1) * 512],
                                lhsT=wv_sb[:, c * 128:(c + 1) * 128],
                                rhs=xT[c][:, n * 512:(n + 1) * 512],
                                start=(c == 0), stop=(c == DC - 1))
                    vt_sb = mp.tile([128, S], F32, tag="vt")
                    nc.vector.tensor_scalar_add(
                        vt_sb[:], psv[:], bias_sb[:, 6:7])

                # V natural (bf16): transpose V^T k-tiles
                with tc.tile_pool(name="vtr", bufs=2, space="PSUM") as vtrp:
                    for g in range(4):
                        pv = vtrp.tile([128, 512], F32, tag="vtr")
                        for i in range(4):
                            kt = g * 4 + i
                            nc.tensor.transpose(
                                pv[:, i * 128:(i + 1) * 128],
                                vt_sb[:, kt * 128:(kt + 1) * 128], ident[:])
                        nc.vector.tensor_copy(V4[g][:], pv[:])

            # ---- Interleaved softmax: A (natural S -> attn) + B (S^T -> z) --
            # Half-tiles [128,1024] ping-pong across two single-buffered PSUM
            # pools + the zz accumulator: 2+2+4 = 8 banks exactly. PE stays
            # dense (HAM warm); ACT paces the loop.
            with tc.tile_pool(name="psA", bufs=1, space="PSUM") as psA, \
                 tc.tile_pool(name="psB", bufs=1, space="PSUM") as psB, \
                 tc.tile_pool(name="zzp", bufs=1, space="PSUM") as zzp, \
                 tc.tile_pool(name="attn", bufs=6) as ap, \
                 tc.tile_pool(name="et", bufs=2) as etp:
                zz = zzp.tile([128, S], F32, tag="zz")
                Z2 = [mp.tile([128, 2 * QT], F32, tag=f"z2{h}", name=f"z2{h}")
                      for h in range(2)]
                for h in range(2):
                    for qt in range(QT):
                        kt = qt
                        at = ap.tile([128, S], F32, tag="at")
                        et = etp.tile([128, S], BF16, tag="et")
                        for half in range(2):
                            lo = half * 1024
                            # A: S[qtile, khalf] -> exp -> attn tile half
                            psa = psA.tile([128, 1024], F32, tag="sA")
                            for j in range(2):
                                n = 2 * half + j
                                nc.tensor.matmul(
                                    psa[:, j * 512:(j + 1) * 512],
                                    lhsT=QQ[h][:, qt * 128:(qt + 1) * 128],
                                    rhs=KK[h][:, n * 512:(n + 1) * 512],
                                    start=True, stop=True)
                            nc.scalar.activation(
                                at[:, lo:lo + 1024], psa[:], EXP, scale=0.125,
                                accum_out=Z2[h][:, 2 * qt + half:2 * qt + half + 1])
                            # B: S^T[ktile, qhalf] -> exp -> E^T half -> AV
                            psb = psB.tile([128, 1024], F32, tag="sB")
                            for j in range(2):
                                n = 2 * half + j
                                nc.tensor.matmul(
                                    psb[:, j * 512:(j + 1) * 512],
                                    lhsT=KK[h][:, kt * 128:(kt + 1) * 128],
                                    rhs=QQ[h][:, n * 512:(n + 1) * 512],
                                    start=True, stop=True)
                            nc.scalar.activation(
                                et[:, lo:lo + 1024], psb[:], EXP, scale=0.125)
                            for j in range(2):
                                n = 2 * half + j
                                nc.tensor.matmul(
                                    zz[h * 64:(h + 1) * 64, n * 512:(n + 1) * 512],
                                    lhsT=V4[kt // 4][:, (kt % 4) * 128 + h * 64:
                                                     (kt % 4) * 128 + h * 64 + 64],
                                    rhs=et[:, n * 512:(n + 1) * 512],
                                    start=(kt == 0), stop=(kt == QT - 1))
                        # combine half-sums, normalize, store attn tile
                        nc.vector.tensor_add(
                            Z_sb[h][:, qt:qt + 1],
                            Z2[h][:, 2 * qt:2 * qt + 1],
                            Z2[h][:, 2 * qt + 1:2 * qt + 2])
                        nc.vector.reciprocal(
                            R_sb[h][:, qt:qt + 1], Z_sb[h][:, qt:qt + 1])
                        nc.vector.tensor_scalar_mul(
                            at[:], at[:], R_sb[h][:, qt:qt + 1])
                        nc.sync.dma_start(
                            out=attn_d[h, qt * 128:(qt + 1) * 128, :], in_=at[:])
                zstg = mp.tile([128, S], F32R, tag="zstg")
                nc.vector.tensor_copy(zz0_sb[:], zz[0:64, :])
                nc.vector.tensor_copy(zstg[64:128, :], zz[64:128, :])
                nc.sync.dma_start(out=zz1_sb[:], in_=zstg[64:128, :])

            # ------- Out projection: unnormalized per head; host scales -----
            with tc.tile_pool(name="psO", bufs=2, space="PSUM") as psO, \
                 tc.tile_pool(name="ot", bufs=4) as otp:
                for h in range(2):
                    nc.sync.dma_start(out=zout_d[:, h * QT:(h + 1) * QT],
                                      in_=Z_sb[h][:])
                for qt in range(QT):
                    po = [psO.tile([128, D], F32, tag=f"po{h}", name=f"po{h}")
                          for h in range(2)]
                    for h, (zsb, wsb, od) in enumerate(
                            ((zz0_sb, wo0_sb, outp0_d),
                             (zz1_sb, wo1_sb, outp1_d))):
                        nc.tensor.matmul(
                            po[h][:],
                            lhsT=zsb[:, qt * 128:(qt + 1) * 128],
                            rhs=wsb[:], start=True, stop=True)
                        ot = otp.tile([128, D], F32, tag=f"ot{h}",
                                      name=f"ot{h}")
                        nc.scalar.copy(ot[:], po[h][:])
                        nc.sync.dma_start(
                            out=od[qt * 128:(qt + 1) * 128, :], in_=ot[:])

    nc.compile()
    return nc


def _get_nc():
    global _CACHED_NC
    if _CACHED_NC is None:
        _CACHED_NC = _build()
    return _CACHED_NC


def _in_maps(emb_x, pe_rel_t, Wq, bq, Wk, bk, Wv, bv, Wo, bo):
    maps = []
    for c in range(8):
        b = c // 4
        h0 = 2 * (c % 4)
        sl = slice(64 * h0, 64 * (h0 + 2))
        bq2, bk2, bv2 = bq[sl], bk[sl], bv[sl]
        bias = np.zeros((128, 7), np.float32)
        bias[0:64, 0] = bq2[0:64]          # Q_c/Q_r bias, head 0 of pair
        bias[0:64, 1] = bq2[64:128]        # head 1 of pair
        bias[0:64, 2] = 2.0 * bk2[0:64]    # K_sum bias h0
        bias[0:64, 3] = bk2[0:64]          # K_c bias h0
        bias[0:64, 4] = 2.0 * bk2[64:128]  # K_sum bias h1
        bias[0:64, 5] = bk2[64:128]        # K_c bias h1
        bias[:, 6] = bv2                   # V bias (both heads)
        maps.append({
            "x": np.ascontiguousarray(emb_x[b].T),
            "pe": np.ascontiguousarray(pe_rel_t[b].T),
            "wq": np.ascontiguousarray(Wq[:, sl]),
            "wk": np.ascontiguousarray(Wk[:, sl]),
            "wv": np.ascontiguousarray(Wv[:, sl]),
            "wo": np.ascontiguousarray(Wo[sl, :]),
            "bias": bias,
        })
    return maps


def _run(in_maps, **kw):
    nc = _get_nc()
    return bass_utils.run_bass_kernel_spmd(nc, in_maps, core_ids=list(range(8)), **kw)


def _assemble(results, bo):
    attn = np.empty((2, 8, S, S), np.float32)
    out = np.empty((2, S, D), np.float32)
    for b in range(2):
        acc = None
        for p in range(4):
            r = results[4 * b + p]
            attn[b, 2 * p:2 * p + 2] = r["attn_sh"]
            z = r["zout"]  # [128, 32]; Z_h[q] at [q % 128, h*16 + q//128]
            part = np.zeros((S, D), np.float32)
            for h, po in enumerate((r["outp0"], r["outp1"])):
                zh = z[:, h * QT:(h + 1) * QT].T.reshape(S)
                part += po / zh[:, None]
            acc = part if acc is None else acc + part
        out[b] = acc + bo[None, :]
    return out, attn


def kernel(emb_x, pe_rel_t, Wq, bq, Wk, bk, Wv, bv, Wo, bo):
    args = [np.asarray(a, np.float32) for a in
            (emb_x, pe_rel_t, Wq, bq, Wk, bk, Wv, bv, Wo, bo)]
    res = _run(_in_maps(*args))
    return _assemble(res.results, args[9])
